# revision 1
# baseline (speedup 1.0000x reference)
"""Trainium2 Bass kernel for nn_Cross_Frequency_Enhanced_Block.

kernel(**inputs) takes FULL unsharded inputs (as in setup_inputs()) and
returns the FULL (32, 1024, 512) float32 output.

Sharding: data-parallel over batch B across 8 NeuronCores (4 batches/core).

Algorithm notes (validated vs reference in numpy, absmax ~4e-6):
  - rfft(x @ Wq.T)[:, :32] == Wq @ rfft(x)[:, :32]: DFT x once per batch via
    matmuls against cos/sin tables (only 32 modes needed), apply Wq/Wk in the
    frequency domain; q/k never materialize in the time domain.
  - complex tanh via the stable sech formula with Cody-Waite range reduction
    for sin/cos (ACT Sin domain is [-pi, pi]).
  - per-mode complex weight einsum: stationary [VR|VI] / [-VI|VR] column
    pairs, moving = w mode-slab (f32r, N=512).
  - irfft as matmul against a (64, 1024) table (1/(D*D) and 2/L folded in).
  - moving average (k=128, edge replicate) via DVE prefix scan + shifted
    differences.  u - mov(u) kills any constant bias exactly, so bo drops.
  - BatchNorm(eval) folded into the final PE-transpose eviction as per-l
    scale/bias on ACT.

Matmul operands are float32r end-to-end (DRAM->SBUF->PE); walrus requires
f32r consumers to see f32r producers.  All engine (DVE/ACT) multi-operand
ops keep every operand at the same start partition.
"""

import os
from contextlib import ExitStack

import numpy as np

import concourse.bacc as bacc
import concourse.bass as bass
import concourse.tile as tile
import concourse.mybir as mybir
from concourse.bass_utils import run_bass_kernel_spmd

B, L, D, MODES = 32, 1024, 512, 32
NCORES = 8
BPC = B // NCORES
F32 = mybir.dt.float32
FR = mybir.dt.float32r
AF = mybir.ActivationFunctionType
ALU = mybir.AluOpType

MAGIC = float(np.float32(12582912.0))        # 1.5*2^23 round-to-nearest
CW1 = float(np.float32(6.28125))             # 2pi hi (exact in f32)
CW2 = float(2 * np.pi - 6.28125)             # 2pi lo
INV2PI = float(np.float32(1.0 / (2 * np.pi)))
PI = float(np.float32(np.pi))
SIM_GELU = bool(int(os.environ.get("BK_SIM_GELU", "0")))


def _tables():
    l_ = np.arange(L)[:, None].astype(np.float64)
    m_ = np.arange(MODES)[None, :].astype(np.float64)
    ang = 2 * np.pi * l_ * m_ / L
    F = np.concatenate([np.cos(ang), -np.sin(ang)], 1).astype(np.float32)
    ftab = np.ascontiguousarray(F.reshape(8, 128, 64).transpose(1, 0, 2))

    a = np.full((MODES,), 2.0 / L)
    a[0] = 1.0 / L
    a = a / (D * D)
    Gc = a[:, None] * np.cos(2 * np.pi * m_.T * l_.T / L)
    Gs = a[:, None] * -np.sin(2 * np.pi * m_.T * l_.T / L)
    gtab = np.concatenate([Gc, Gs], 0).astype(np.float32)

    ident = np.eye(128, dtype=np.float32)
    coefF = np.ascontiguousarray(np.broadcast_to(
        64.0 - np.arange(65, dtype=np.float32), (128, 65)))
    coefE = np.ascontiguousarray(np.broadcast_to(
        np.arange(63, dtype=np.float32) + 1.0, (128, 63)))
    return ftab, gtab, ident, coefF, coefE


def _t128(w):
    """(512, 512) host array -> (128, 4, 512) [p, ch, col] with row=ch*128+p."""
    return np.ascontiguousarray(w.reshape(4, 128, 512).transpose(1, 0, 2))


def _build(iters=1):
    nc = bacc.Bacc("TRN2", target_bir_lowering=False, debug=False,
                   num_devices=NCORES)
    dram = {}

    def din(name, shape, dt=FR):
        dram[name] = nc.dram_tensor(name, list(shape), dt,
                                    kind="ExternalInput").ap()

    MLOC = MODES // NCORES               # modes owned per core
    din("xs", (BPC, L, D))
    din("wslab", (MLOC, 2, D, D))        # per-core mode slice of w
    for n in ("wqt", "wkt", "wot", "w1t", "w2t"):
        din(n, (128, 4, D))
    din("ftab", (128, 8, 64))
    din("gtab", (64, L))
    din("identt", (128, 128))
    din("coefF", (128, 65))
    din("coefE", (128, 63))
    din("bnt", (128, 8, 4), F32)
    din("bqkt", (128, 4, 2), F32)
    out_d = nc.dram_tensor("out", [BPC, L, D], F32, kind="ExternalOutput").ap()
    # collective staging: xqkv -> mode owners, einsum result -> batch owners
    vq_dI = nc.dram_tensor("vq_d", [iters, BPC, 64, D], FR).ap()
    vq_sndI = nc.dram_tensor("vq_snd", [iters, NCORES, BPC, 2, MLOC, D], FR).ap()
    vq_rcvI = nc.dram_tensor("vq_rcv", [iters, NCORES, BPC, 2, MLOC, D], FR).ap()
    xwm_dI = nc.dram_tensor("xwm_d", [iters, MLOC, 64, D], FR).ap()
    xw_sndI = nc.dram_tensor("xw_snd", [iters, NCORES, MLOC, 2, BPC, D], FR).ap()
    xw_rcvI = nc.dram_tensor("xw_rcv", [iters, NCORES, MLOC, 2, BPC, D], FR).ap()

    with tile.TileContext(nc) as tc, ExitStack() as ctx:
        con = ctx.enter_context(tc.tile_pool(name="con", bufs=1))
        wrk = ctx.enter_context(tc.tile_pool(name="wrk", bufs=1))
        big = ctx.enter_context(tc.tile_pool(name="big", bufs=1))
        wpool = ctx.enter_context(tc.tile_pool(name="wpool", bufs=6))
        outp = ctx.enter_context(tc.tile_pool(name="outp", bufs=2))
        ps = ctx.enter_context(tc.tile_pool(name="ps", bufs=4, space="PSUM"))
        ps2 = ctx.enter_context(tc.tile_pool(name="ps2", bufs=2, space="PSUM"))

        def cload(name, shape, dt=FR):
            t = con.tile(list(shape), dt, tag=name)
            nc.sync.dma_start(out=t[:], in_=dram[name])
            return t

        ftab = cload("ftab", (128, 8, 64))
        gtab = cload("gtab", (64, L))
        ident = cload("identt", (128, 128))
        wqt = cload("wqt", (128, 4, D))
        wkt = cload("wkt", (128, 4, D))
        wot = cload("wot", (128, 4, D))
        w1t = cload("w1t", (128, 4, D))
        w2t = cload("w2t", (128, 4, D))
        coefF = cload("coefF", (128, 65))
        coefE = cload("coefE", (128, 63))
        bnt = cload("bnt", (128, 8, 4), F32)
        bqkt = cload("bqkt", (128, 4, 2), F32)

        def tt(o, a, bb, op):
            nc.vector.tensor_tensor(o, a, bb, op)

        # --- BN constants: c1 = gamma*rsqrt(var+eps), c0 = beta - mean*c1
        c1l = con.tile([128, 8], F32)
        c0l = con.tile([128, 8], F32)
        sq = con.tile([128, 8], F32)
        ve = con.tile([128, 8], F32)
        yy = con.tile([128, 8], F32)
        nc.vector.tensor_scalar_add(ve[:], bnt[:, :, 3], 1e-5)
        nc.scalar.activation(sq[:], ve[:], AF.Sqrt)
        nc.vector.reciprocal(c1l[:], sq[:])
        tt(yy[:], c1l[:], c1l[:], ALU.mult)
        tt(yy[:], yy[:], ve[:], ALU.mult)
        nc.vector.tensor_scalar(yy[:], yy[:], -0.5, 1.5, ALU.mult, ALU.add)
        tt(c1l[:], c1l[:], yy[:], ALU.mult)
        tt(c1l[:], c1l[:], bnt[:, :, 0], ALU.mult)
        tt(c0l[:], bnt[:, :, 2], c1l[:], ALU.mult)
        tt(c0l[:], bnt[:, :, 1], c0l[:], ALU.subtract)
        bqs = con.tile([128, 4, 2], FR)
        nc.scalar.activation(bqs[:], bqkt[:], AF.Copy, scale=float(L))

        for _it in range(iters):
            vq_d = vq_dI[_it]
            vq_snd = vq_sndI[_it]
            vq_rcv = vq_rcvI[_it]
            xwm_d = xwm_dI[_it]
            xw_snd = xw_sndI[_it]
            xw_rcv = xw_rcvI[_it]
            # =================== stage A: x load + 32-mode DFT =================
            xf_all = wrk.tile([128, 4, BPC, 64], FR, tag="mid2")
            for b in range(BPC):
                xt = big.tile([128, 8, D], FR, tag="xx")
                nc.sync.dma_start(
                    out=xt[:],
                    in_=dram["xs"][b].rearrange("(t p) d -> p t d", p=128))
                xfT_ps = ps.tile([128, 512], F32, tag="ps")
                for lt in range(8):
                    nc.tensor.matmul(xfT_ps[0:64, :], ftab[:, lt, :],
                                     xt[:, lt, :],
                                     start=(lt == 0), stop=(lt == 7))
                xfT_sb = wrk.tile([64, 512], FR, tag="xfT")
                nc.vector.tensor_copy(xfT_sb[:], xfT_ps[0:64, :])
                xf_ps = ps.tile([128, 512], FR, tag="ps")
                for dch in range(4):
                    nc.tensor.transpose(xf_ps[:, dch * 64:dch * 64 + 64],
                                        xfT_sb[:, dch * 128:dch * 128 + 128],
                                        ident[0:64, 0:64])
                nc.vector.tensor_copy(
                    xf_all[:, :, b, :],
                    xf_ps[:, 0:256].rearrange("p (c m) -> p c m", c=4))

            # =================== stage B: qf/kf in frequency domain ============
            qkf = wrk.tile([128, 4, BPC, 128], FR, tag="mid")
            for wt, co in ((wqt, 0), (wkt, 64)):
                for ech in range(4):
                    qp = ps.tile([128, 512], F32, tag="ps")
                    for dch in range(4):
                        nc.tensor.matmul(
                            qp[:, 0:256], wt[:, dch, ech * 128:ech * 128 + 128],
                            xf_all[:, dch, :, :],
                            start=(dch == 0), stop=(dch == 3))
                    nc.vector.tensor_copy(
                        qkf[:, ech, :, co:co + 64],
                        qp[:, 0:256].rearrange("p (b m) -> p b m", b=BPC))
            # bias: mode-0 real += L*b  (DFT of constant vector)
            for ech in range(4):
                for co, j in ((0, 0), (64, 1)):
                    tt(qkf[:, ech, :, co:co + 1], qkf[:, ech, :, co:co + 1],
                       bqs[:, ech:ech + 1, j:j + 1].to_broadcast([128, BPC, 1]),
                       ALU.add)

            # =================== stage C: Z, tanh, U, xqkv =====================
            # Z split into Re/Im row blocks so every engine op stays at start
            # partition 0:  ZR[x, y'] = sum_e qfRe[e, x] kf[e, y'], ZI likewise.
            ZpsR = ps.tile([32, 512], F32, tag="ps")
            ZpsI = ps.tile([32, 512], F32, tag="ps")
            for b in range(BPC):
                for ech in range(4):
                    nc.tensor.matmul(
                        ZpsR[0:32, b * 64:b * 64 + 64],
                        qkf[:, ech, b, 0:32], qkf[:, ech, b, 64:128],
                        start=(ech == 0), stop=(ech == 3))
                    nc.tensor.matmul(
                        ZpsI[0:32, b * 64:b * 64 + 64],
                        qkf[:, ech, b, 32:64], qkf[:, ech, b, 64:128],
                        start=(ech == 0), stop=(ech == 3))
            ZsbR = wrk.tile([32, BPC, 64], F32)
            ZsbI = wrk.tile([32, BPC, 64], F32)
            nc.vector.tensor_copy(
                ZsbR[:], ZpsR[0:32, 0:256].rearrange("p (b y) -> p b y", b=BPC))
            nc.vector.tensor_copy(
                ZsbI[:], ZpsI[0:32, 0:256].rearrange("p (b y) -> p b y", b=BPC))

            sh = [32, BPC, 32]
            zr = wrk.tile(sh, F32)
            zi = wrk.tile(sh, F32)
            # Z = (QR + iQI).(KR + iKI):  Re = QR.KR - QI.KI, Im = QR.KI + QI.KR
            tt(zr[:], ZsbR[:, :, 0:32], ZsbI[:, :, 32:64], ALU.subtract)
            tt(zi[:], ZsbR[:, :, 32:64], ZsbI[:, :, 0:32], ALU.add)
            tht = wrk.tile(sh, F32)
            sech = wrk.tile(sh, F32)
            s2y = wrk.tile(sh, F32)
            c2y = wrk.tile(sh, F32)
            w1 = wrk.tile(sh, F32)
            w2 = wrk.tile(sh, F32)
            w3 = wrk.tile(sh, F32)
            nc.scalar.activation(tht[:], zr[:], AF.Tanh, scale=2.0)
            nc.scalar.activation(w1[:], zr[:], AF.Abs, scale=2.0)
            nc.vector.tensor_scalar_min(w1[:], w1[:], 87.0)
            nc.scalar.activation(w1[:], w1[:], AF.Exp, scale=-1.0)   # e^-2|x|
            tt(w2[:], w1[:], w1[:], ALU.mult)
            nc.vector.tensor_scalar_add(w2[:], w2[:], 1.0)
            nc.vector.reciprocal(w2[:], w2[:])
            tt(sech[:], w1[:], w2[:], ALU.mult)
            nc.vector.tensor_scalar(sech[:], sech[:], 2.0, None, ALU.mult)
            for dst, ofs in ((s2y, 0.0), (c2y, 0.25)):
                # k = round(2*zi/(2pi) + ofs) via the 1.5*2^23 magic-add trick
                nc.vector.tensor_scalar(w1[:], zi[:], 2.0 * INV2PI, MAGIC + ofs,
                                        ALU.mult, ALU.add)
                nc.vector.tensor_scalar_sub(w1[:], w1[:], MAGIC)
                # red = 2*zi (+ pi/2 for cos) - k*CW1 - k*CW2, clamp to [-pi, pi]
                nc.vector.tensor_scalar(w2[:], zi[:], 2.0, ofs * 2.0 * PI,
                                        ALU.mult, ALU.add)
                nc.vector.tensor_scalar(w3[:], w1[:], CW1, None, ALU.mult)
                tt(w2[:], w2[:], w3[:], ALU.subtract)
                nc.vector.tensor_scalar(w3[:], w1[:], CW2, None, ALU.mult)
                tt(w2[:], w2[:], w3[:], ALU.subtract)
                nc.vector.tensor_scalar(w2[:], w2[:], -PI, PI, ALU.max, ALU.min)
                nc.scalar.activation(dst[:], w2[:], AF.Sin)
            tt(w1[:], c2y[:], sech[:], ALU.mult)
            nc.vector.tensor_scalar_add(w1[:], w1[:], 1.0)
            nc.vector.reciprocal(w1[:], w1[:])                       # 1/den
            TR = wrk.tile(sh, FR)
            TI = wrk.tile(sh, FR)
            tt(TR[:], tht[:], w1[:], ALU.mult)
            tt(TI[:], s2y[:], sech[:], ALU.mult)
            tt(TI[:], TI[:], w1[:], ALU.mult)
            # U1 = [TR^T | TI^T], U2 = [-TI^T | TR^T] per batch (start part 0)
            U1 = wrk.tile([32, BPC, 64], FR)
            U2 = wrk.tile([32, BPC, 64], FR)
            for b in range(BPC):
                tp1 = ps.tile([32, 128], FR, tag="ps")
                nc.tensor.transpose(tp1[0:32, 0:32], TR[:, b, :],
                                    ident[0:32, 0:32])
                nc.tensor.transpose(tp1[0:32, 64:96], TI[:, b, :],
                                    ident[0:32, 0:32])
                nc.vector.tensor_copy(U1[:, b, 0:32], tp1[0:32, 0:32])
                nc.vector.tensor_copy(U1[:, b, 32:64], tp1[0:32, 64:96])
                nc.scalar.activation(U2[:, b, 0:32], tp1[0:32, 64:96],
                                     AF.Copy, scale=-1.0)
                nc.vector.tensor_copy(U2[:, b, 32:64], tp1[0:32, 0:32])
            # xqkv per b -> transpose to (col, e) and stage to DRAM for AllToAll
            for b in range(BPC):
                kpsR = ps.tile([32, 512], FR, tag="ps")
                kpsI = ps.tile([32, 512], FR, tag="ps")
                for ech in range(4):
                    nc.tensor.transpose(kpsR[0:32, ech * 128:ech * 128 + 128],
                                        qkf[:, ech, b, 64:96], ident[:])
                    nc.tensor.transpose(kpsI[0:32, ech * 128:ech * 128 + 128],
                                        qkf[:, ech, b, 96:128], ident[:])
                kfTR = wrk.tile([32, 512], FR, tag="kfTR")
                kfTI = wrk.tile([32, 512], FR, tag="kfTI")
                nc.vector.tensor_copy(kfTR[:], kpsR[0:32, :])
                nc.vector.tensor_copy(kfTI[:], kpsI[0:32, :])
                vps = ps.tile([128, 512], F32, tag="ps")
                for ech in range(4):
                    nc.tensor.matmul(vps[:, ech * 64:ech * 64 + 64],
                                     kfTR[:, ech * 128:ech * 128 + 128],
                                     U1[:, b, :], start=True, stop=False)
                    nc.tensor.matmul(vps[:, ech * 64:ech * 64 + 64],
                                     kfTI[:, ech * 128:ech * 128 + 128],
                                     U2[:, b, :], start=False, stop=True)
                vsb = wrk.tile([128, 4, 64], FR, tag="kfTI2")
                nc.vector.tensor_copy(
                    vsb[:], vps[:, 0:256].rearrange("p (c m) -> p c m", c=4))
                vTp = ps.tile([64, 512], FR, tag="ps")
                for ech in range(4):
                    nc.tensor.transpose(vTp[0:64, ech * 128:ech * 128 + 128],
                                        vsb[:, ech, :], ident[:])
                vT_sb = wrk.tile([64, 512], FR, tag="vT")
                nc.vector.tensor_copy(vT_sb[:], vTp[0:64, :])
                nc.sync.dma_start(out=vq_d[b], in_=vT_sb[:])

            # =================== stage D: AllToAll + mode-sharded einsum =======
            # exchange 1: route each core's xqkv columns to the mode owner
            grp = [list(range(NCORES))]
            nc.sync.dma_start(
                out=vq_snd[:],
                in_=vq_d.rearrange("b (k j m) e -> j b k m e", k=2, j=NCORES,
                                   m=MLOC))
            nc.gpsimd.collective_compute(
                "AllToAll", ALU.bypass, replica_groups=grp,
                ins=[vq_snd.opt()], outs=[vq_rcv.opt()])
            # xqgT: partition p = kind*32 + b_global, free (m_local, e)
            xqgT = wrk.tile([64, MLOC, D], FR, tag="mid")
            for k in range(2):
                nc.sync.dma_start(
                    out=xqgT[k * 32:k * 32 + 32, :, :],
                    in_=vq_rcv[:, :, k, :, :].rearrange("s b m e -> (s b) m e"))
            # transpose back to (e-part, cols=(kind, b_global)) per (m', ech)
            xqa = wrk.tile([128, 4, MLOC, 64], FR, tag="mid2")
            xqa2 = wrk.tile([128, 4, MLOC, 64], FR, tag="qkve2")
            for ml in range(MLOC):
                xp = ps.tile([128, 512], FR, tag="ps")
                for ech in range(4):
                    nc.tensor.transpose(xp[:, ech * 64:ech * 64 + 64],
                                        xqgT[0:64, ml, ech * 128:ech * 128 + 128],
                                        ident[0:64, 0:64])
                nc.vector.tensor_copy(
                    xqa[:, :, ml, :],
                    xp[:, 0:256].rearrange("p (c m) -> p c m", c=4))
            nc.scalar.activation(xqa2[:, :, :, 0:32], xqa[:, :, :, 32:64],
                                 AF.Copy, scale=-1.0)
            nc.vector.tensor_copy(xqa2[:, :, :, 32:64], xqa[:, :, :, 0:32])
            _nmodes = 0 if os.environ.get("BK_SKIP_D") else MLOC
            with tc.tile_pool(name="pse", bufs=2, space="PSUM") as pse:
                for ml in range(_nmodes):
                    pm = pse.tile([64, 512], F32, tag="pm")
                    for ech in range(4):
                        wR = wpool.tile([128, 512], FR, tag="wt")
                        nc.sync.dma_start(
                            out=wR[:],
                            in_=dram["wslab"][ml, 0, ech * 128:ech * 128 + 128, :])
                        wI = wpool.tile([128, 512], FR, tag="wt")
                        nc.sync.dma_start(
                            out=wI[:],
                            in_=dram["wslab"][ml, 1, ech * 128:ech * 128 + 128, :])
                        nc.tensor.matmul(pm[:], xqa[:, ech, ml, :], wR,
                                         start=(ech == 0), stop=False)
                        nc.tensor.matmul(pm[:], xqa2[:, ech, ml, :], wI,
                                         start=False, stop=(ech == 3))
                    xw_sb = outp.tile([64, 512], FR, tag="mid3")
                    nc.vector.tensor_copy(xw_sb[:], pm[:])
                    nc.sync.dma_start(out=xwm_d[ml], in_=xw_sb[:])
            # exchange 2: route per-mode results back to batch owners
            nc.sync.dma_start(
                out=xw_snd[:],
                in_=xwm_d.rearrange("m (r j bl) o -> j m r bl o", r=2, j=NCORES,
                                    bl=BPC))
            nc.gpsimd.collective_compute(
                "AllToAll", ALU.bypass, replica_groups=grp,
                ins=[xw_snd.opt()], outs=[xw_rcv.opt()])

            # =================== stage E: irfft, Wo, MA, convs, BN =============
            _bpce = 0 if os.environ.get("BK_SKIP_E") else BPC
            for b in range(_bpce):
                XXT = wrk.tile([64, 512], FR, tag="xxt")
                for r in range(2):
                    nc.sync.dma_start(
                        out=XXT[r * 32:r * 32 + 32, :],
                        in_=xw_rcv[:, :, r, b, :].rearrange("s m o -> (s m) o"))
                fre = big.tile([128, 4, L], FR, tag="e2")
                for och in range(4):
                    for lh in range(2):
                        fp = ps2.tile([128, 512], F32, tag="ps2")
                        nc.tensor.matmul(fp[:],
                                         XXT[:, och * 128:och * 128 + 128],
                                         gtab[:, lh * 512:lh * 512 + 512],
                                         start=True, stop=True)
                        nc.vector.tensor_copy(fre[:, och, lh * 512:lh * 512 + 512],
                                              fp[:])
                # reload x and transpose it to (d-part, l) for the residual
                xe = big.tile([128, 8, D], FR, tag="xx")
                nc.sync.dma_start(
                    out=xe[:],
                    in_=dram["xs"][b].rearrange("(t p) d -> p t d", p=128))
                xT = big.tile([128, 4, L], FR, tag="e4")
                for dch in range(4):
                    for lh in range(2):
                        tp0 = ps.tile([128, 512], FR, tag="ps")
                        for lq in range(4):
                            lt = lh * 4 + lq
                            nc.tensor.transpose(
                                tp0[:, lq * 128:lq * 128 + 128],
                                xe[:, lt, dch * 128:dch * 128 + 128], ident[:])
                        if (dch + lh) % 2 == 0:
                            nc.vector.tensor_copy(
                                xT[:, dch, lh * 512:lh * 512 + 512], tp0[:])
                        else:
                            nc.scalar.copy(
                                xT[:, dch, lh * 512:lh * 512 + 512], tp0[:])
                u_s = big.tile([128, 4, L], FR, tag="e5")
                for dch in range(4):
                    for lh in range(2):
                        up = ps.tile([128, 512], F32, tag="ps")
                        for och in range(4):
                            nc.tensor.matmul(
                                up[:], wot[:, och, dch * 128:dch * 128 + 128],
                                fre[:, och, lh * 512:lh * 512 + 512],
                                start=(och == 0), stop=False)
                        nc.tensor.matmul(
                            up[:], ident[:],
                            xT[:, dch, lh * 512:lh * 512 + 512],
                            start=False, stop=True)
                        nc.scalar.copy(u_s[:, dch, lh * 512:lh * 512 + 512], up[:])
                cs = big.tile([128, 4, L], FR, tag="e4")
                for dch in range(4):
                    nc.vector.tensor_tensor_scan(
                        cs[:, dch, :], u_s[:, dch, :], u_s[:, dch, :], 0.0,
                        ALU.add, ALU.bypass)
                tmov = big.tile([128, 4, L], FR, tag="e1")
                ef = wrk.tile([128, 65], FR, tag="ef")
                ee_ = wrk.tile([128, 63], FR, tag="ee")
                e2_ = wrk.tile([128, 63], FR, tag="e2s")
                for dch in range(4):
                    tt(tmov[:, dch, 65:961], cs[:, dch, 128:1024],
                       cs[:, dch, 0:896], ALU.subtract)
                    tt(ef[:], u_s[:, dch, 0:1].to_broadcast([128, 65]), coefF[:],
                       ALU.mult)
                    tt(tmov[:, dch, 0:65], cs[:, dch, 63:128], ef[:], ALU.add)
                    tt(ee_[:], u_s[:, dch, 1023:1024].to_broadcast([128, 63]),
                       coefE[:], ALU.mult)
                    tt(e2_[:], cs[:, dch, 1023:1024].to_broadcast([128, 63]),
                       cs[:, dch, 896:959], ALU.subtract)
                    tt(tmov[:, dch, 961:1024], ee_[:], e2_[:], ALU.add)
                # x_dec = u - mov(u), computed in place into u_s
                nc.scalar.activation(tmov[:], tmov[:], AF.Copy, scale=1.0 / 128)
                tt(u_s[:], u_s[:], tmov[:], ALU.subtract)
                xd = u_s
                y1g = big.tile([128, 4, L], FR, tag="e2")
                for och in range(4):
                    for lh in range(2):
                        cp = ps2.tile([128, 512], F32, tag="ps2")
                        for dch in range(4):
                            nc.tensor.matmul(
                                cp[:], w1t[:, dch, och * 128:och * 128 + 128],
                                xd[:, dch, lh * 512:lh * 512 + 512],
                                start=(dch == 0), stop=(dch == 3))
                        yslc = y1g[:, och, lh * 512:lh * 512 + 512]
                        if not SIM_GELU:
                            nc.scalar.activation(yslc, cp[:], AF.Gelu)
                        else:
                            # CoreSim has no Gelu LUT: tanh-approx stand-in
                            y1c = wrk.tile([128, 512], F32, tag="gel1")
                            nc.scalar.copy(y1c[:], cp[:])
                            sqt = wrk.tile([128, 512], F32, tag="gel2")
                            nc.scalar.activation(sqt[:], y1c[:], AF.Square)
                            tt(sqt[:], sqt[:], y1c[:], ALU.mult)
                            nc.vector.tensor_scalar(sqt[:], sqt[:], 0.044715,
                                                    None, ALU.mult)
                            tt(sqt[:], sqt[:], y1c[:], ALU.add)
                            nc.vector.tensor_scalar(sqt[:], sqt[:],
                                                    0.7978845608028654,
                                                    None, ALU.mult)
                            nc.scalar.activation(sqt[:], sqt[:], AF.Tanh)
                            nc.vector.tensor_scalar(sqt[:], sqt[:], 0.5, 0.5,
                                                    ALU.mult, ALU.add)
                            tt(yslc, y1c[:], sqt[:], ALU.mult)
                res = big.tile([128, 4, L], FR, tag="e1")
                for dch in range(4):
                    for lh in range(2):
                        rp = ps.tile([128, 512], F32, tag="ps")
                        for och in range(4):
                            nc.tensor.matmul(
                                rp[:], w2t[:, och, dch * 128:dch * 128 + 128],
                                y1g[:, och, lh * 512:lh * 512 + 512],
                                start=(och == 0), stop=False)
                        nc.tensor.matmul(
                            rp[:], ident[:],
                            xd[:, dch, lh * 512:lh * 512 + 512],
                            start=False, stop=True)
                        nc.vector.tensor_copy(
                            res[:, dch, lh * 512:lh * 512 + 512], rp[:])
                for lt in range(8):
                    tp = ps.tile([128, 512], FR, tag="ps")
                    for dch in range(4):
                        nc.tensor.transpose(
                            tp[:, dch * 128:dch * 128 + 128],
                            res[:, dch, lt * 128:lt * 128 + 128], ident[:])
                    ob = outp.tile([128, 512], F32, tag="ob")
                    nc.scalar.activation(ob[:], tp[:], AF.Identity,
                                         bias=c0l[:, lt:lt + 1],
                                         scale=c1l[:, lt:lt + 1])
                    nc.sync.dma_start(out=out_d[b, lt * 128:lt * 128 + 128, :],
                                      in_=ob[:])

    nc.compile()
    return nc


_CACHE = {}


def _get_nc(iters=1):
    key = f"nc{iters}"
    if key not in _CACHE:
        _CACHE[key] = _build(iters)
    return _CACHE[key]


def _host_inputs(inputs):
    x = np.ascontiguousarray(inputs["x"], dtype=np.float32)
    ftab, gtab, ident, coefF, coefE = _tables()
    wr = np.asarray(inputs["w_real"], dtype=np.float32)[0]   # (E, O, MODES)
    wi = np.asarray(inputs["w_imag"], dtype=np.float32)[0]
    wslab = np.ascontiguousarray(
        np.stack([wr.transpose(2, 0, 1), wi.transpose(2, 0, 1)], axis=1))
    bn = [np.asarray(inputs[k], dtype=np.float32)
          for k in ("bn_gamma", "bn_beta", "bn_mean", "bn_var")]
    bnt = np.ascontiguousarray(
        np.stack(bn, -1).reshape(8, 128, 4).transpose(1, 0, 2))
    bq = np.asarray(inputs["bq"], dtype=np.float32)
    bk = np.asarray(inputs["bk"], dtype=np.float32)
    bqkt = np.ascontiguousarray(
        np.stack([bq.reshape(4, 128).T, bk.reshape(4, 128).T], -1))
    com = {
        "wqt": _t128(np.asarray(inputs["Wq"], np.float32).T),
        "wkt": _t128(np.asarray(inputs["Wk"], np.float32).T),
        "wot": _t128(np.asarray(inputs["Wo"], np.float32).T),
        "w1t": _t128(np.asarray(inputs["conv1_w"], np.float32).T),
        "w2t": _t128(np.asarray(inputs["conv2_w"], np.float32).T),
        "ftab": ftab, "gtab": gtab, "identt": ident,
        "coefF": coefF, "coefE": coefE, "bnt": bnt, "bqkt": bqkt,
    }
    mloc = MODES // NCORES
    maps = []
    for c in range(NCORES):
        m = dict(com)
        m["xs"] = np.ascontiguousarray(x[c * BPC:(c + 1) * BPC])
        m["wslab"] = np.ascontiguousarray(wslab[c * mloc:(c + 1) * mloc])
        maps.append(m)
    return maps


def kernel(**inputs):
    nc = _get_nc()
    in_maps = _host_inputs(inputs)
    trace = bool(int(os.environ.get("BK_TRACE", "0")))
    res = run_bass_kernel_spmd(nc, in_maps, core_ids=list(range(NCORES)),
                               trace=trace)
    if trace and res.exec_time_ns is not None:
        print(f"HW exec time: {res.exec_time_ns} ns")
        _CACHE["exec_time_ns"] = res.exec_time_ns
    out = np.concatenate([res.results[c]["out"] for c in range(NCORES)], 0)
    return out.astype(np.float32)



# revision 23
# speedup vs baseline: 1.3966x; 1.3966x over previous
"""Trainium2 Bass kernel for nn_Cross_Frequency_Enhanced_Block.

kernel(**inputs) takes FULL unsharded inputs (as in setup_inputs()) and
returns the FULL (32, 1024, 512) float32 output.

Sharding: data-parallel over batch B across 8 NeuronCores (4 batches/core);
the per-mode complex-weight einsum is mode-sharded (4 modes/core) with two
AllToAll exchanges.

Algorithm notes (validated vs reference):
  - rfft(x @ Wq.T)[:, :32] == Wq @ rfft(x)[:, :32]: DFT x once per batch via
    matmuls against cos/sin tables (only 32 modes needed), apply Wq/Wk in the
    frequency domain; q/k never materialize in the time domain.
  - complex tanh via the stable sech formula with Cody-Waite range reduction
    for sin/cos (ACT Sin domain is [-pi, pi]).
  - irfft as matmul against a (64, 1024) table (1/(D*D) and 2/L folded in).
  - moving average (k=128, edge replicate) via prefix scan + shifted
    differences.  u - mov(u) kills any constant bias exactly, so bo drops.
  - BatchNorm(eval) folded into the final PE-transpose eviction as per-l
    scale/bias on ACT.

Performance structure (v2):
  - bf16 for weights and most intermediates (validated ~6e-3 rel err, limit
    2e-2); the residual stream u_s/xd and final BN are computed with a single
    bf16 rounding per site.
  - software-pipelined stage E: PE stream per step s is
    irfft(s), Wo(s), conv1(s-1), conv2(s-1), outT(s-1), xT(s+2)-trans,
    so the serial moving-average chain of batch s-1 (DVE+Pool) hides under
    batch s's matmuls.  All big tiles live in 2-deep rings.
  - residual adds fused into PSUM evictions via scalar_tensor_tensor on
    DVE/Pool (no identity matmuls on PE).
  - moving average: cumsum scans + window diffs split across DVE and Pool
    (2 channels each), diffs staged in an f32 scratch so the bf16 residual
    tile is rounded only once.
  - DMA queues: x / exchange staging / outputs on SP; the 4 MiB bf16 mode-
    weight stream + XXT on ACT's queue, issued at iteration start so the
    slabs arrive before the einsum needs them.
  - exchange staging writes go straight to the AllToAll send buffer with a
    rearranged DRAM view (no DRAM->DRAM repack).
"""

import os
from contextlib import ExitStack

import numpy as np
import ml_dtypes

import concourse.bacc as bacc
import concourse.bass as bass
import concourse.tile as tile
import concourse.mybir as mybir
from concourse.bass_utils import run_bass_kernel_spmd

B, L, D, MODES = 32, 1024, 512, 32
NCORES = 8
BPC = B // NCORES
MLOC = MODES // NCORES
F32 = mybir.dt.float32
FR = mybir.dt.float32r
BF = mybir.dt.bfloat16
AF = mybir.ActivationFunctionType
ALU = mybir.AluOpType

MAGIC = float(np.float32(12582912.0))        # 1.5*2^23 round-to-nearest
CW1 = float(np.float32(6.28125))             # 2pi hi (exact in f32)
CW2 = float(2 * np.pi - 6.28125)             # 2pi lo
INV2PI = float(np.float32(1.0 / (2 * np.pi)))
PI = float(np.float32(np.pi))
SIM_GELU = bool(int(os.environ.get("BK_SIM_GELU", "0")))
NBF = ml_dtypes.bfloat16


def _tables():
    l_ = np.arange(L)[:, None].astype(np.float64)
    m_ = np.arange(MODES)[None, :].astype(np.float64)
    ang = 2 * np.pi * l_ * m_ / L
    F = np.concatenate([np.cos(ang), -np.sin(ang)], 1).astype(np.float32)
    ftab = np.ascontiguousarray(F.reshape(8, 128, 64).transpose(1, 0, 2))

    a = np.full((MODES,), 2.0 / L)
    a[0] = 1.0 / L
    a = a / (D * D)
    Gc = a[:, None] * np.cos(2 * np.pi * m_.T * l_.T / L)
    Gs = a[:, None] * -np.sin(2 * np.pi * m_.T * l_.T / L)
    gtab = np.concatenate([Gc, Gs], 0).astype(NBF)

    ident = np.eye(128, dtype=np.float32)
    identb = np.eye(128, dtype=NBF)
    coefF = np.ascontiguousarray(np.broadcast_to(
        64.0 - np.arange(65, dtype=np.float32), (128, 65)))
    coefE = np.ascontiguousarray(np.broadcast_to(
        np.arange(63, dtype=np.float32) + 1.0, (128, 63)))
    return ftab, gtab, ident, identb, coefF, coefE


def _t128(w):
    """(512, 512) host array -> (128, 4, 512) [p, ch, col] with row=ch*128+p."""
    return np.ascontiguousarray(w.reshape(4, 128, 512).transpose(1, 0, 2))


def _build(iters=1, local_coll=False):
    nc = bacc.Bacc("TRN2", target_bir_lowering=False, debug=False,
                   num_devices=NCORES)
    dram = {}

    def din(name, shape, dt=FR):
        dram[name] = nc.dram_tensor(name, list(shape), dt,
                                    kind="ExternalInput").ap()

    din("xs", (BPC, L, D))
    din("wslab", (MLOC, 2, D, D), BF)     # per-core mode slice of w
    for n in ("wqt", "wkt", "wot", "w1t", "w2t"):
        din(n, (128, 4, D), BF)
    din("ftab", (128, 8, 64))
    din("gtab", (64, L), BF)
    din("identt", (128, 128))
    din("identbt", (128, 128), BF)
    din("coefF", (128, 65), F32)
    din("coefE", (128, 63), F32)
    din("bnt", (128, 8, 4), F32)
    din("bqkt", (128, 4, 2), F32)
    out_d = nc.dram_tensor("out", [BPC, L, D], F32, kind="ExternalOutput").ap()
    # collective staging: xqkv -> mode owners, einsum result -> batch owners
    vq_dI = nc.dram_tensor("vq_d", [iters, BPC, 64, D], BF).ap()
    xwm_dI = nc.dram_tensor("xwm_d", [iters, MLOC, 64, D], BF).ap()
    vq_sndI = nc.dram_tensor("vq_snd", [iters, NCORES, BPC, 2, MLOC, D], BF).ap()
    vq_rcvI = nc.dram_tensor("vq_rcv", [iters, NCORES, BPC, 2, MLOC, D], BF).ap()
    xw_sndI = nc.dram_tensor("xw_snd", [iters, NCORES, MLOC, 2, BPC, D], BF).ap()
    xw_rcvI = nc.dram_tensor("xw_rcv", [iters, NCORES, MLOC, 2, BPC, D], BF).ap()

    with tile.TileContext(nc) as tc, ExitStack() as ctx:
        con = ctx.enter_context(tc.tile_pool(name="con", bufs=1))
        wrk = ctx.enter_context(tc.tile_pool(name="wrk", bufs=1))
        rng = ctx.enter_context(tc.tile_pool(name="rng", bufs=1))
        psm = ctx.enter_context(tc.tile_pool(name="psm", bufs=1, space="PSUM"))

        def cload(name, shape, dt=FR):
            t = con.tile(list(shape), dt, tag=name, name=name)
            nc.scalar.dma_start(out=t[:], in_=dram[name])
            return t

        # stage A needs come first in the ACT DMA queue
        ftab = cload("ftab", (128, 8, 64))
        ident = cload("identt", (128, 128))
        identb = cload("identbt", (128, 128), BF)
        wqt = cload("wqt", (128, 4, D), BF)
        wkt = cload("wkt", (128, 4, D), BF)
        gtab = cload("gtab", (64, L), BF)
        wot = cload("wot", (128, 4, D), BF)
        w1t = cload("w1t", (128, 4, D), BF)
        w2t = cload("w2t", (128, 4, D), BF)
        coefF = cload("coefF", (128, 65), F32)
        coefE = cload("coefE", (128, 63), F32)
        bnt = cload("bnt", (128, 8, 4), F32)
        bqkt = cload("bqkt", (128, 4, 2), F32)

        def tt(o, a, bb, op, eng=None):
            (eng or nc.vector).tensor_tensor(o, a, bb, op)

        # --- BN constants: c1 = gamma*rsqrt(var+eps), c0 = beta - mean*c1
        c1l = con.tile([128, 8], F32)
        c0l = con.tile([128, 8], F32)
        sq = con.tile([128, 8], F32)
        ve = con.tile([128, 8], F32)
        yy = con.tile([128, 8], F32)
        nc.vector.tensor_scalar_add(ve[:], bnt[:, :, 3], 1e-5)
        nc.scalar.activation(sq[:], ve[:], AF.Sqrt)
        nc.vector.reciprocal(c1l[:], sq[:])
        tt(yy[:], c1l[:], c1l[:], ALU.mult)
        tt(yy[:], yy[:], ve[:], ALU.mult)
        nc.vector.tensor_scalar(yy[:], yy[:], -0.5, 1.5, ALU.mult, ALU.add)
        tt(c1l[:], c1l[:], yy[:], ALU.mult)
        tt(c1l[:], c1l[:], bnt[:, :, 0], ALU.mult)
        tt(c0l[:], bnt[:, :, 2], c1l[:], ALU.mult)
        tt(c0l[:], bnt[:, :, 1], c0l[:], ALU.subtract)
        bqs = con.tile([128, 4, 2], F32)
        nc.scalar.activation(bqs[:], bqkt[:], AF.Copy, scale=float(L))

        grp = [list(range(NCORES))]
        EVAL = [nc.vector, nc.gpsimd]      # round-robin evict engines

        for _it in range(iters):
            vq_d = vq_dI[_it]
            xwm_d = xwm_dI[_it]
            vq_snd = vq_sndI[_it]
            vq_rcv = vq_rcvI[_it]
            xw_snd = xw_sndI[_it]
            xw_rcv = xw_rcvI[_it]

            # ========== phase 0: bulk DMA issues ==========
            # mode-weight stream on ACT queue; full ring so no WAR stalls.
            wsl = {}

            def load_wsl(ml, ri):
                t = rng.tile([128, 4, D], BF, tag="wsl", bufs=4, name="wsl")
                nc.scalar.dma_start(
                    out=t[:],
                    in_=dram["wslab"][ml, ri].rearrange(
                        "(c p) o -> p c o", p=128))
                wsl[(ml, ri)] = t

            for ml in range(2):
                for ri in range(2):
                    load_wsl(ml, ri)

            # ========== stage A: x load + 32-mode DFT ==========
            xf_all = wrk.tile([128, 4, BPC, 64], BF, tag="xfa")
            xts = []
            for b in range(BPC):
                halves = []
                for h in range(2):
                    xth = rng.tile([128, 4, D], FR, tag="xx", bufs=3,
                                   name="xth")
                    nc.sync.dma_start(
                        out=xth[:],
                        in_=dram["xs"][b, h * 512:h * 512 + 512].rearrange(
                            "(t p) d -> p t d", p=128))
                    halves.append(xth)
                xts.append(halves)
            xTs = [None] * BPC
            for b in range(BPC):
                xt = xts[b]
                psA = psm.tile([128, 512], F32, tag="mm", bufs=3, name="psA")
                for lt in range(8):
                    nc.tensor.matmul(psA[0:64, :], ftab[:, lt, :],
                                     xt[lt // 4][:, lt % 4, :],
                                     start=(lt == 0), stop=(lt == 7))
                xfT = wrk.tile([64, 512], BF, tag="xfT", bufs=2, name="xfT")
                nc.scalar.copy(xfT[:], psA[0:64, :])
                psT = psm.tile([128, 512], BF, tag="tb", bufs=2, name="psT")
                for dch in range(4):
                    nc.tensor.transpose(psT[:, dch * 64:dch * 64 + 64],
                                        xfT[:, dch * 128:dch * 128 + 128],
                                        identb[0:64, 0:64])
                nc.vector.tensor_copy(
                    xf_all[:, :, b, :],
                    psT[:, 0:256].rearrange("p (c m) -> p c m", c=4))
                # xT(b) = x transposed to (d-part, l), bf16, for stage E
                xT = rng.tile([128, 4, L], BF, tag="xT", bufs=4, name="xT")
                for dch in range(4):
                    for lh in range(2):
                        tp0 = psm.tile([128, 512], FR, tag="tp", bufs=2,
                                       name="tp0")
                        for lq in range(4):
                            nc.tensor.transpose(
                                tp0[:, lq * 128:lq * 128 + 128],
                                xt[lh][:, lq, dch * 128:dch * 128 + 128],
                                ident[:])
                        nc.scalar.copy(xT[:, dch, lh * 512:lh * 512 + 512],
                                       tp0[:])
                xTs[b] = xT

            # ========== stage B: qf/kf in frequency domain ==========
            qkf = wrk.tile([128, 4, BPC, 128], BF, tag="qkf")
            for wt, co in ((wqt, 0), (wkt, 64)):
                for ech in range(4):
                    qp = psm.tile([128, 512], F32, tag="mm", bufs=3,
                                  name="qp")
                    for dch in range(4):
                        nc.tensor.matmul(
                            qp[:, 0:256],
                            wt[:, dch, ech * 128:ech * 128 + 128],
                            xf_all[:, dch, :, :],
                            start=(dch == 0), stop=(dch == 3))
                    nc.vector.tensor_copy(
                        qkf[:, ech, :, co:co + 64],
                        qp[:, 0:256].rearrange("p (b m) -> p b m", b=BPC))
            # bias: mode-0 real += L*b  (DFT of constant vector)
            for ech in range(4):
                for co, j in ((0, 0), (64, 1)):
                    tt(qkf[:, ech, :, co:co + 1], qkf[:, ech, :, co:co + 1],
                       bqs[:, ech:ech + 1, j:j + 1].to_broadcast([128, BPC, 1]),
                       ALU.add, eng=nc.gpsimd)

            # ========== stage C: Z, tanh, U, xqkv ==========
            ZpsR = psm.tile([32, 512], F32, tag="mm", bufs=3, name="ZpsR")
            ZpsI = psm.tile([32, 512], F32, tag="mm", bufs=3, name="ZpsI")
            for b in range(BPC):
                for ech in range(4):
                    nc.tensor.matmul(
                        ZpsR[0:32, b * 64:b * 64 + 64],
                        qkf[:, ech, b, 0:32], qkf[:, ech, b, 64:128],
                        start=(ech == 0), stop=(ech == 3))
                    nc.tensor.matmul(
                        ZpsI[0:32, b * 64:b * 64 + 64],
                        qkf[:, ech, b, 32:64], qkf[:, ech, b, 64:128],
                        start=(ech == 0), stop=(ech == 3))
            ZsbR = wrk.tile([32, BPC, 64], F32)
            ZsbI = wrk.tile([32, BPC, 64], F32)
            nc.vector.tensor_copy(
                ZsbR[:], ZpsR[0:32, 0:256].rearrange("p (b y) -> p b y", b=BPC))
            nc.vector.tensor_copy(
                ZsbI[:], ZpsI[0:32, 0:256].rearrange("p (b y) -> p b y", b=BPC))

            sh = [32, BPC, 32]
            zr = wrk.tile(sh, F32)
            zi = wrk.tile(sh, F32)
            # Z = (QR + iQI).(KR + iKI):  Re = QR.KR - QI.KI, Im = QR.KI + QI.KR
            tt(zr[:], ZsbR[:, :, 0:32], ZsbI[:, :, 32:64], ALU.subtract)
            tt(zi[:], ZsbR[:, :, 32:64], ZsbI[:, :, 0:32], ALU.add,
               eng=nc.gpsimd)
            tht = wrk.tile(sh, F32)
            sech = wrk.tile(sh, F32)
            s2y = wrk.tile(sh, F32)
            c2y = wrk.tile(sh, F32)
            w1 = wrk.tile(sh, F32)
            w2 = wrk.tile(sh, F32)
            nc.scalar.activation(tht[:], zr[:], AF.Tanh, scale=2.0)
            nc.scalar.activation(w1[:], zr[:], AF.Abs, scale=2.0)
            nc.vector.tensor_scalar_min(w1[:], w1[:], 87.0)
            nc.scalar.activation(w1[:], w1[:], AF.Exp, scale=-1.0)   # e^-2|x|
            tt(w2[:], w1[:], w1[:], ALU.mult)
            nc.vector.tensor_scalar_add(w2[:], w2[:], 1.0)
            nc.vector.reciprocal(w2[:], w2[:])
            tt(sech[:], w1[:], w2[:], ALU.mult)
            nc.vector.tensor_scalar(sech[:], sech[:], 2.0, None, ALU.mult)
            # sin/cos branches on separate engines (independent chains)
            for dst, ofs, eng in ((s2y, 0.0, nc.vector), (c2y, 0.25, nc.gpsimd)):
                v1 = wrk.tile(sh, F32, tag=f"v1{ofs}", name="v1")
                v2 = wrk.tile(sh, F32, tag=f"v2{ofs}", name="v2")
                v3 = wrk.tile(sh, F32, tag=f"v3{ofs}", name="v3")
                # k = round(2*zi/(2pi) + ofs) via the 1.5*2^23 magic-add trick
                eng.tensor_scalar(v1[:], zi[:], 2.0 * INV2PI, MAGIC + ofs,
                                  ALU.mult, ALU.add)
                eng.tensor_scalar(v1[:], v1[:], MAGIC, None, ALU.subtract)
                # red = 2*zi (+ pi/2 for cos) - k*CW1 - k*CW2, clamp [-pi, pi]
                eng.tensor_scalar(v2[:], zi[:], 2.0, ofs * 2.0 * PI,
                                  ALU.mult, ALU.add)
                eng.tensor_scalar(v3[:], v1[:], CW1, None, ALU.mult)
                tt(v2[:], v2[:], v3[:], ALU.subtract, eng=eng)
                eng.tensor_scalar(v3[:], v1[:], CW2, None, ALU.mult)
                tt(v2[:], v2[:], v3[:], ALU.subtract, eng=eng)
                eng.tensor_scalar(v2[:], v2[:], -PI, PI, ALU.max, ALU.min)
                nc.scalar.activation(dst[:], v2[:], AF.Sin)
            tt(w1[:], c2y[:], sech[:], ALU.mult)
            nc.vector.tensor_scalar_add(w1[:], w1[:], 1.0)
            nc.vector.reciprocal(w1[:], w1[:])                       # 1/den
            TR = wrk.tile(sh, BF)
            TI = wrk.tile(sh, BF)
            tt(TR[:], tht[:], w1[:], ALU.mult)
            tt(TI[:], s2y[:], sech[:], ALU.mult, eng=nc.gpsimd)
            tt(TI[:], TI[:], w1[:], ALU.mult, eng=nc.gpsimd)
            # U1 = [TR^T | TI^T], U2 = [-TI^T | TR^T] per batch (start part 0)
            U1 = wrk.tile([32, BPC, 64], BF)
            U2 = wrk.tile([32, BPC, 64], BF)
            for b in range(BPC):
                tp1 = psm.tile([32, 128], BF, tag="tb", bufs=2, name="tp1")
                nc.tensor.transpose(tp1[0:32, 0:32], TR[:, b, :],
                                    identb[0:32, 0:32])
                nc.tensor.transpose(tp1[0:32, 64:96], TI[:, b, :],
                                    identb[0:32, 0:32])
                nc.vector.tensor_copy(U1[:, b, 0:32], tp1[0:32, 0:32])
                nc.vector.tensor_copy(U1[:, b, 32:64], tp1[0:32, 64:96])
                nc.scalar.activation(U2[:, b, 0:32], tp1[0:32, 64:96],
                                     AF.Copy, scale=-1.0)
                nc.vector.tensor_copy(U2[:, b, 32:64], tp1[0:32, 0:32])
            # xqkv: all k-transposes first (no PE head-of-line on evicts),
            # then per-b matmul/transpose software-pipelined
            kfTRs, kfTIs = [], []
            for b in range(BPC):
                kpsR = psm.tile([32, 512], BF, tag="tb", bufs=2, name="kpsR")
                kpsI = psm.tile([32, 512], BF, tag="tb", bufs=2, name="kpsI")
                for ech in range(4):
                    nc.tensor.transpose(kpsR[0:32, ech * 128:ech * 128 + 128],
                                        qkf[:, ech, b, 64:96], identb[:])
                    nc.tensor.transpose(kpsI[0:32, ech * 128:ech * 128 + 128],
                                        qkf[:, ech, b, 96:128], identb[:])
                kfTR = wrk.tile([32, 512], BF, tag="kfTR", bufs=2, name="kfTR")
                kfTI = wrk.tile([32, 512], BF, tag="kfTI", bufs=2, name="kfTI")
                nc.vector.tensor_copy(kfTR[:], kpsR[0:32, :])
                nc.vector.tensor_copy(kfTI[:], kpsI[0:32, :])
                kfTRs.append(kfTR)
                kfTIs.append(kfTI)
            vsbs = []
            for b in range(BPC):
                vps = psm.tile([128, 512], F32, tag="mm", bufs=3, name="vps")
                for ech in range(4):
                    nc.tensor.matmul(vps[:, ech * 64:ech * 64 + 64],
                                     kfTRs[b][:, ech * 128:ech * 128 + 128],
                                     U1[:, b, :], start=True, stop=False)
                    nc.tensor.matmul(vps[:, ech * 64:ech * 64 + 64],
                                     kfTIs[b][:, ech * 128:ech * 128 + 128],
                                     U2[:, b, :], start=False, stop=True)
                vsb = wrk.tile([128, 4, 64], BF, tag="vsb", bufs=2, name="vsb")
                nc.vector.tensor_copy(
                    vsb[:], vps[:, 0:256].rearrange("p (c m) -> p c m", c=4))
                vsbs.append(vsb)
            for b in range(BPC):
                vTp = psm.tile([64, 512], BF, tag="tb", bufs=2, name="vTp")
                for ech in range(4):
                    nc.tensor.transpose(vTp[0:64, ech * 128:ech * 128 + 128],
                                        vsbs[b][:, ech, :], identb[:])
                vT_sb = wrk.tile([64, 512], BF, tag="vT", bufs=2, name="vT")
                nc.scalar.copy(vT_sb[:], vTp[0:64, :])
                nc.sync.dma_start(out=vq_d[b], in_=vT_sb[:])

            # ========== exchange 1 ==========
            nc.sync.dma_start(
                out=vq_snd[:],
                in_=vq_d.rearrange("b (k j m) e -> j b k m e", k=2, j=NCORES,
                                   m=MLOC))
            if local_coll:
                nc.sync.dma_start(out=vq_rcv[:], in_=vq_snd[:])
            else:
                nc.gpsimd.collective_compute(
                    "AllToAll", ALU.bypass, replica_groups=grp,
                    ins=[vq_snd.opt()], outs=[vq_rcv.opt()])

            # ========== stage D: mode-sharded einsum ==========
            # xqgT: partition p = kind*32 + b_global, free (m_local, e)
            xqgT = wrk.tile([64, MLOC, D], BF, tag="xqgT")
            for k in range(2):
                nc.sync.dma_start(
                    out=xqgT[k * 32:k * 32 + 32, :, :],
                    in_=vq_rcv[:, :, k, :, :].rearrange("s b m e -> (s b) m e"))
            # transpose back to (e-part, cols=(kind, b_global)) per (m', ech)
            xqa = wrk.tile([128, 4, MLOC, 64], BF, tag="xqa")
            xqa2 = wrk.tile([128, 4, MLOC, 64], BF, tag="xqa2")
            for ml in range(MLOC):
                xp = psm.tile([128, 512], BF, tag="tb", bufs=2, name="xp")
                for ech in range(4):
                    nc.tensor.transpose(xp[:, ech * 64:ech * 64 + 64],
                                        xqgT[0:64, ml, ech * 128:ech * 128 + 128],
                                        identb[0:64, 0:64])
                nc.vector.tensor_copy(
                    xqa[:, :, ml, :],
                    xp[:, 0:256].rearrange("p (c m) -> p c m", c=4))
            nc.scalar.activation(xqa2[:, :, :, 0:32], xqa[:, :, :, 32:64],
                                 AF.Copy, scale=-1.0)
            nc.gpsimd.tensor_copy(xqa2[:, :, :, 32:64], xqa[:, :, :, 0:32])
            for ml in range(2, 4):
                for ri in range(2):
                    load_wsl(ml, ri)
            for ml in range(MLOC):
                pm = psm.tile([64, 512], F32, tag="mm", bufs=3, name="pm")
                for ech in range(4):
                    nc.tensor.matmul(pm[:], xqa[:, ech, ml, :],
                                     wsl[(ml, 0)][:, ech, :],
                                     start=(ech == 0), stop=False)
                    nc.tensor.matmul(pm[:], xqa2[:, ech, ml, :],
                                     wsl[(ml, 1)][:, ech, :],
                                     start=False, stop=(ech == 3))
                xw_sb = wrk.tile([64, 512], BF, tag="xwsb", bufs=2, name="xwsb")
                nc.vector.tensor_copy(xw_sb[:], pm[:])
                nc.sync.dma_start(out=xwm_d[ml], in_=xw_sb[:])

            # ========== exchange 2 ==========
            nc.sync.dma_start(
                out=xw_snd[:],
                in_=xwm_d.rearrange("m (r j bl) o -> j m r bl o", r=2,
                                    j=NCORES, bl=BPC))
            if local_coll:
                nc.sync.dma_start(out=xw_rcv[:], in_=xw_snd[:])
            else:
                nc.gpsimd.collective_compute(
                    "AllToAll", ALU.bypass, replica_groups=grp,
                    ins=[xw_snd.opt()], outs=[xw_rcv.opt()])

            # XXT loads (tiny, ACT queue)
            XXTs = []
            for b in range(BPC):
                XXT = wrk.tile([64, 512], BF, tag="XXT", bufs=4, name="XXT")
                for r in range(2):
                    nc.scalar.dma_start(
                        out=XXT[r * 32:r * 32 + 32, :],
                        in_=xw_rcv[:, :, r, b, :].rearrange(
                            "s m o -> (s m) o"))
                XXTs.append(XXT)

            # ========== stage E: software-pipelined over batches ==========
            fres = [None] * BPC
            uss = [None] * BPC
            ress = [None] * BPC

            def emit_head(s):
                """irfft(s) + Wo(s) with fused residual-add eviction."""
                XXT = XXTs[s]
                fre = rng.tile([128, 4, L], BF, tag="fy", bufs=2, name="fre")
                fres[s] = fre
                k = 0
                for och in range(4):
                    for lh in range(2):
                        fp = psm.tile([128, 512], F32, tag="mm", bufs=3,
                                      name="fp")
                        nc.tensor.matmul(fp[:],
                                         XXT[:, och * 128:och * 128 + 128],
                                         gtab[:, lh * 512:lh * 512 + 512],
                                         start=True, stop=True)
                        nc.scalar.copy(fre[:, och, lh * 512:lh * 512 + 512],
                                       fp[:])
                        k += 1
                us = rng.tile([128, 4, L], BF, tag="us", bufs=2, name="us")
                uss[s] = us
                xT = xTs[s]
                for dch in range(4):
                    for lh in range(2):
                        up = psm.tile([128, 512], F32, tag="mm", bufs=3,
                                      name="up")
                        for och in range(4):
                            nc.tensor.matmul(
                                up[:], wot[:, och, dch * 128:dch * 128 + 128],
                                fre[:, och, lh * 512:lh * 512 + 512],
                                start=(och == 0), stop=(och == 3))
                        nc.vector.scalar_tensor_tensor(
                            us[:, dch, lh * 512:lh * 512 + 512], up[:], 1.0,
                            xT[:, dch, lh * 512:lh * 512 + 512],
                            ALU.mult, ALU.add)
                        k += 1

            def emit_ma(s):
                """moving-average update of us[s] in place; cs scan + window
                diffs split across DVE (dch 0,1) and Pool (dch 2,3)."""
                us = uss[s]
                cs = rng.tile([128, 4, L], F32, tag="cs", bufs=1, name="cs")
                # scans + scalar_tensor_tensor are DVE-only (Pool/gpsimd has
                # no InstTensorScalarPtr / scan support); Pool covers two
                # channels' interiors with plain tt/ts passes.
                for dch in range(4):
                    nc.vector.tensor_tensor_scan(
                        cs[:, dch, :], us[:, dch, :], us[:, dch, :], 0.0,
                        ALU.add, ALU.bypass)
                masP = rng.tile([128, L], F32, tag="masP", bufs=1, name="masP")
                for dch in (2, 3):
                    # interior on Pool: m = cs_hi - cs_lo; m *= -1/128; u += m
                    nc.gpsimd.tensor_tensor(
                        masP[:, 0:896], cs[:, dch, 128:1024],
                        cs[:, dch, 0:896], ALU.subtract)
                    nc.gpsimd.tensor_scalar(
                        masP[:, 0:896], masP[:, 0:896], -1.0 / 128, None,
                        ALU.mult)
                    nc.gpsimd.tensor_tensor(
                        us[:, dch, 65:961], us[:, dch, 65:961], masP[:, 0:896],
                        ALU.add)
                mas = rng.tile([128, L], F32, tag="mas", bufs=1, name="mas")
                for dch in range(4):
                    m = mas
                    if dch < 2:
                        # interior on DVE (fused)
                        nc.vector.tensor_tensor(
                            m[:, 0:896], cs[:, dch, 128:1024],
                            cs[:, dch, 0:896], ALU.subtract)
                        nc.vector.scalar_tensor_tensor(
                            us[:, dch, 65:961], m[:, 0:896], -1.0 / 128,
                            us[:, dch, 65:961], ALU.mult, ALU.add)
                    # edges on DVE for all channels
                    # front: mov*128 = cs[63:128] + u0*coefF
                    nc.vector.scalar_tensor_tensor(
                        m[:, 896:961], coefF[:], us[:, dch, 0:1],
                        cs[:, dch, 63:128], ALU.mult, ALU.add)
                    nc.vector.scalar_tensor_tensor(
                        us[:, dch, 0:65], m[:, 896:961], -1.0 / 128,
                        us[:, dch, 0:65], ALU.mult, ALU.add)
                    # end: mov*128 = uL*coefE + (cs[1023] - cs[896:959])
                    nc.vector.scalar_tensor_tensor(
                        m[:, 961:1024], cs[:, dch, 896:959],
                        cs[:, dch, 1023:1024], cs[:, dch, 896:959],
                        ALU.subtract, ALU.bypass)
                    nc.vector.scalar_tensor_tensor(
                        m[:, 961:1024], coefE[:], us[:, dch, 1023:1024],
                        m[:, 961:1024], ALU.mult, ALU.subtract)
                    nc.vector.scalar_tensor_tensor(
                        us[:, dch, 961:1024], m[:, 961:1024], -1.0 / 128,
                        us[:, dch, 961:1024], ALU.mult, ALU.add)

            def emit_tail(s):
                """conv1(s) -> gelu -> conv2(s) + residual -> BN transpose
                -> out DMA."""
                xd = uss[s]
                y1g = rng.tile([128, 4, L], BF, tag="fy", bufs=2, name="y1g")
                k = 0
                for och in range(4):
                    for lh in range(2):
                        cp = psm.tile([128, 512], F32, tag="mm", bufs=3,
                                      name="cp")
                        for dch in range(4):
                            nc.tensor.matmul(
                                cp[:], w1t[:, dch, och * 128:och * 128 + 128],
                                xd[:, dch, lh * 512:lh * 512 + 512],
                                start=(dch == 0), stop=(dch == 3))
                        yslc = y1g[:, och, lh * 512:lh * 512 + 512]
                        if not SIM_GELU:
                            nc.scalar.activation(yslc, cp[:], AF.Gelu)
                        else:
                            # CoreSim has no Gelu LUT: tanh-approx stand-in
                            y1c = wrk.tile([128, 512], F32, tag="gel1",
                                           name="y1c")
                            nc.scalar.copy(y1c[:], cp[:])
                            sqt = wrk.tile([128, 512], F32, tag="gel2",
                                           name="sqt")
                            nc.scalar.activation(sqt[:], y1c[:], AF.Square)
                            tt(sqt[:], sqt[:], y1c[:], ALU.mult)
                            nc.vector.tensor_scalar(sqt[:], sqt[:], 0.044715,
                                                    None, ALU.mult)
                            tt(sqt[:], sqt[:], y1c[:], ALU.add)
                            nc.vector.tensor_scalar(sqt[:], sqt[:],
                                                    0.7978845608028654,
                                                    None, ALU.mult)
                            nc.scalar.activation(sqt[:], sqt[:], AF.Tanh)
                            nc.vector.tensor_scalar(sqt[:], sqt[:], 0.5, 0.5,
                                                    ALU.mult, ALU.add)
                            tt(yslc, y1c[:], sqt[:], ALU.mult)
                res = rng.tile([128, 4, L], BF, tag="res", bufs=1, name="res")
                ress[s] = res
                for dch in range(4):
                    for lh in range(2):
                        rp = psm.tile([128, 512], F32, tag="mm", bufs=3,
                                      name="rp")
                        for och in range(4):
                            nc.tensor.matmul(
                                rp[:], w2t[:, och, dch * 128:dch * 128 + 128],
                                y1g[:, och, lh * 512:lh * 512 + 512],
                                start=(och == 0), stop=False)
                        nc.tensor.matmul(
                            rp[:], identb[:],
                            xd[:, dch, lh * 512:lh * 512 + 512],
                            start=False, stop=True)
                        nc.scalar.copy(res[:, dch, lh * 512:lh * 512 + 512],
                                       rp[:])
                        k += 1
                for lt in range(8):
                    tp = psm.tile([128, 512], BF, tag="tb", bufs=2, name="tp")
                    for dch in range(4):
                        nc.tensor.transpose(
                            tp[:, dch * 128:dch * 128 + 128],
                            res[:, dch, lt * 128:lt * 128 + 128], identb[:])
                    ob = rng.tile([128, 512], F32, tag="ob", bufs=2, name="ob")
                    nc.scalar.activation(ob[:], tp[:], AF.Identity,
                                         bias=c0l[:, lt:lt + 1],
                                         scale=c1l[:, lt:lt + 1])
                    nc.sync.dma_start(out=out_d[s, lt * 128:lt * 128 + 128, :],
                                      in_=ob[:])

            for s in range(BPC + 1):
                if s >= 1:
                    emit_ma(s - 1)
                if s < BPC:
                    emit_head(s)
                if s >= 1:
                    emit_tail(s - 1)

    nc.compile()
    return nc


_CACHE = {}


def _get_nc(iters=1):
    key = f"nc{iters}"
    if key not in _CACHE:
        _CACHE[key] = _build(
            iters, local_coll=bool(int(os.environ.get("BK_LOCAL_COLL", "0"))))
    return _CACHE[key]


def _host_inputs(inputs):
    x = np.ascontiguousarray(inputs["x"], dtype=np.float32)
    ftab, gtab, ident, identb, coefF, coefE = _tables()
    wr = np.asarray(inputs["w_real"], dtype=np.float32)[0]   # (E, O, MODES)
    wi = np.asarray(inputs["w_imag"], dtype=np.float32)[0]
    wslab = np.ascontiguousarray(
        np.stack([wr.transpose(2, 0, 1), wi.transpose(2, 0, 1)],
                 axis=1)).astype(NBF)
    bn = [np.asarray(inputs[k], dtype=np.float32)
          for k in ("bn_gamma", "bn_beta", "bn_mean", "bn_var")]
    bnt = np.ascontiguousarray(
        np.stack(bn, -1).reshape(8, 128, 4).transpose(1, 0, 2))
    bq = np.asarray(inputs["bq"], dtype=np.float32)
    bk = np.asarray(inputs["bk"], dtype=np.float32)
    bqkt = np.ascontiguousarray(
        np.stack([bq.reshape(4, 128).T, bk.reshape(4, 128).T], -1))
    com = {
        "wqt": _t128(np.asarray(inputs["Wq"], np.float32).T).astype(NBF),
        "wkt": _t128(np.asarray(inputs["Wk"], np.float32).T).astype(NBF),
        "wot": _t128(np.asarray(inputs["Wo"], np.float32).T).astype(NBF),
        "w1t": _t128(np.asarray(inputs["conv1_w"], np.float32).T).astype(NBF),
        "w2t": _t128(np.asarray(inputs["conv2_w"], np.float32).T).astype(NBF),
        "ftab": ftab, "gtab": gtab, "identt": ident, "identbt": identb,
        "coefF": coefF, "coefE": coefE, "bnt": bnt, "bqkt": bqkt,
    }
    maps = []
    for c in range(NCORES):
        m = dict(com)
        m["xs"] = np.ascontiguousarray(x[c * BPC:(c + 1) * BPC])
        m["wslab"] = np.ascontiguousarray(wslab[c * MLOC:(c + 1) * MLOC])
        maps.append(m)
    return maps


def kernel(**inputs):
    nc = _get_nc()
    in_maps = _host_inputs(inputs)
    trace = bool(int(os.environ.get("BK_TRACE", "0")))
    res = run_bass_kernel_spmd(nc, in_maps, core_ids=list(range(NCORES)),
                               trace=trace)
    if trace and res.exec_time_ns is not None:
        print(f"HW exec time: {res.exec_time_ns} ns")
        _CACHE["exec_time_ns"] = res.exec_time_ns
    out = np.concatenate([res.results[c]["out"] for c in range(NCORES)], 0)
    return out.astype(np.float32)
